# revision 3
# baseline (speedup 1.0000x reference)
"""CoarseWarp Trainium2 kernel.

Reference computation (shapes hardcoded):
  lr [2,64,64,64] (shape-only), ref [2,64,254,254], index_map [2,64516]
  padded = reflect-pad(ref, 1)                      # [2,64,256,256]
  patches(l) = padded[:, :, y'+i, x'+j], l' = index_map[b, l]
  out[b,c,y+i,x+j] += padded[b,c,y'+i,x'+j]         # 3x3 overlap-add fold
  out: [2,64,256,256] f32

Strategy (8 NeuronCores, pure data parallel):
  core k handles b = k//4, output rows [ (k%4)*64, +64 ).
  Image stored channels-last in HBM: img[pixel=y*256+x, c] — one pixel row
  = 64 f32 = 256 B.  For each l=(y,x) and tap row t, one 768 B chunk
  img[(y'+t)*256+x' : +3 px] is fetched with dma_gather (elem_step=64,
  elem_size=192, int16 indices over two 32768-pixel segments), then
  dma_scatter_add performs the 3x3 overlap-add directly into the per-core
  output slab in HBM (CCE accumulate).  Chunks are sorted by
  (color=x%3, segment) so each scatter instruction has disjoint windows
  (no RMW races) and each gather instruction uses a single segment base.
  Streams are padded to cross-core-max capacities so all 8 cores run one
  SPMD program; pad chunks scatter into a dump row that is dropped.
"""

import numpy as np

B, C = 2, 64
HR = WR = 254
HO = WO = 256
L = HR * WR            # 64516
NPIX = HO * WO         # 65536
SEGPIX = 32768         # pixels per gather segment (int16 reach)
ES = 192               # gather/scatter elem_size (3 px x 64 ch)
STEP = 64              # elem_step (1 px = 256 B)
N_CORES = 8
CORES_PER_LAUNCH = 4   # SWDGE-heavy cores per concurrent launch
SLAB = 64              # output rows per core
HALFR = 32             # rows per half-slab unit
DUMP_IDX = SLAB * WO   # scatter dump target (row 64 of the 65-row slab)


def _wrap16(stream: np.ndarray) -> np.ndarray:
    """Index stream [N] (N%16==0) -> ucode layout [128, N//16] int16.

    Stream position n lives at partition n%16, column n//16, replicated
    across the eight 16-partition groups."""
    a = stream.reshape(-1, 16).T.astype(np.int16)
    return np.tile(a, (8, 1))


def _build_streams(index_map: np.ndarray):
    """Per-core gather/scatter index streams, padded to shared capacities.

    Returns (caps, gidx_cores, sidx_cores):
      caps[t][h][c][s] = shared 128-multiple capacity of group (color c, seg s)
      gidx_cores[k] = [128, GCOLS] int16, sidx_cores[k] = [128, SCOLS] int16
    """
    yp = (index_map // WR).astype(np.int64)   # [B, L]
    xp = (index_map % WR).astype(np.int64)

    xs = np.arange(WR)
    # chunk lists per (core, t, h): dict -> (gidx_raw, seg, color, sidx_raw)
    raw = {}
    counts = np.zeros((N_CORES, 3, 2, 3, 2), np.int64)  # [core,t,h,c,s]
    for k in range(N_CORES):
        b, r0 = k // 4, (k % 4) * SLAB
        for t in range(3):
            for h in range(2):
                rows = np.arange(r0 + HALFR * h, r0 + HALFR * (h + 1))
                ys = rows - t
                ys = ys[(ys >= 0) & (ys < HR)]
                Y = np.repeat(ys, WR)
                X = np.tile(xs, len(ys))
                lin = Y * WR + X
                g = (yp[b, lin] + t) * HO + xp[b, lin]   # gather pixel
                seg = (g >= SEGPIX).astype(np.int64)
                gi = g - seg * SEGPIX
                si = (Y + t - r0) * WO + X               # local scatter pixel
                col = X % 3
                raw[(k, t, h)] = (gi, seg, col, si)
                for c in range(3):
                    for s in range(2):
                        counts[k, t, h, c, s] = np.sum((col == c) & (seg == s))

    caps = np.zeros((3, 2, 3, 2), np.int64)
    for t in range(3):
        for h in range(2):
            for c in range(3):
                for s in range(2):
                    m = counts[:, t, h, c, s].max()
                    caps[t, h, c, s] = ((m + 127) // 128) * 128

    gidx_cores, sidx_cores = [], []
    for k in range(N_CORES):
        gblocks, sblocks = [], []
        for t in range(3):
            for h in range(2):
                gi, seg, col, si = raw[(k, t, h)]
                for c in range(3):
                    sidx_color = []
                    for s in range(2):
                        cap = int(caps[t, h, c, s])
                        sel = (col == c) & (seg == s)
                        gsel = gi[sel]
                        ssel = si[sel]
                        npad = cap - len(gsel)
                        gstream = np.concatenate(
                            [gsel, np.zeros(npad, np.int64)])
                        sstream = np.concatenate(
                            [ssel, np.full(npad, DUMP_IDX, np.int64)])
                        gblocks.append(_wrap16(gstream))
                        sidx_color.append(sstream)
                    sblocks.append(_wrap16(np.concatenate(sidx_color)))
        gidx_cores.append(np.concatenate(gblocks, axis=1))
        sidx_cores.append(np.concatenate(sblocks, axis=1))
    return caps, gidx_cores, sidx_cores


DBG_TH_LIMIT = 6      # how many (t,h) units to emit (debug knob)
DBG_NO_SCATTER = False
DBG_NO_GATHER = False


def _build_program(caps):
    """Build the SPMD Bacc program (shared by all 8 cores)."""
    import bass_rust
    import concourse.bacc as bacc
    import concourse.bass as bass
    import concourse.tile as tile
    from concourse import mybir
    from concourse.library_config import mlp

    gcols_total = int(caps.sum()) // 16
    scols_total = gcols_total

    nc = bacc.Bacc(
        "TRN2",
        target_bir_lowering=False,
        debug=False,
        enable_asserts=False,
        num_devices=N_CORES,
    )
    img_t = nc.dram_tensor(
        "img", [NPIX, C], mybir.dt.float32, kind="ExternalInput")
    gidx_t = nc.dram_tensor(
        "gidx", [128, gcols_total], mybir.dt.int16, kind="ExternalInput")
    sidx_t = nc.dram_tensor(
        "sidx", [128, scols_total], mybir.dt.int16, kind="ExternalInput")
    out_t = nc.dram_tensor(
        "out", [SLAB + 1, WO, C], mybir.dt.float32, kind="ExternalOutput")

    # gather source APs: two overlapping-window segment views
    seg_aps = []
    for s in range(2):
        ap = img_t[s * SEGPIX:, :].copy() if s else img_t[:, :].copy()
        ap.ap = bass_rust.VecI64Pair([[STEP, SEGPIX - 2], [1, ES]])
        seg_aps.append(ap)

    # scatter dest AP: overlapping windows over the whole slab (incl. dump row)
    dst_ap = out_t[:, :, :].copy()
    dst_ap.ap = bass_rust.VecI64Pair([[STEP, (SLAB + 1) * WO - 2], [1, ES]])

    with tile.TileContext(nc) as tc:
        with tc.tile_pool(name="sbuf", bufs=2) as pool, \
             tc.tile_pool(name="idx", bufs=1) as idxpool:
            nc.gpsimd.load_library(mlp)

            gi_sb = idxpool.tile([128, gcols_total], mybir.dt.int16)
            nc.sync.dma_start(out=gi_sb[:], in_=gidx_t[:, :])
            si_sb = idxpool.tile([128, scols_total], mybir.dt.int16)
            nc.sync.dma_start(out=si_sb[:], in_=sidx_t[:, :])

            gofs = 0   # int16 columns consumed in gi_sb
            sofs = 0   # int16 columns consumed in si_sb
            for t in range(3):
                for h in range(2):
                    if t * 2 + h >= DBG_TH_LIMIT:
                        break
                    th_cols = int(caps[t, h].sum()) // 128
                    S = pool.tile([128, th_cols, ES], mybir.dt.float32)
                    MAXI = 1280  # max indices per SWDGE instruction (HW limit)
                    colofs = 0
                    for c in range(3):
                        for s in range(2):
                            cap = int(caps[t, h, c, s])
                            done = 0
                            while done < cap:
                                sub = min(MAXI, cap - done)
                                if not DBG_NO_GATHER:
                                    nc.gpsimd.dma_gather(
                                        S[:, colofs:colofs + sub // 128, :],
                                        seg_aps[s],
                                        gi_sb[:, gofs:gofs + sub // 16],
                                        sub, sub, ES, elem_step=STEP,
                                        single_packet=False,
                                    )
                                colofs += sub // 128
                                gofs += sub // 16
                                done += sub
                    colofs = 0
                    for c in range(3):
                        cap_c = int(caps[t, h, c, 0] + caps[t, h, c, 1])
                        done = 0
                        while done < cap_c:
                            sub = min(MAXI, cap_c - done)
                            if not (DBG_NO_SCATTER or DBG_NO_GATHER):
                                nc.gpsimd.dma_scatter_add(
                                    dst_ap,
                                    S[:, colofs:colofs + sub // 128, :],
                                    si_sb[:, sofs:sofs + sub // 16],
                                    sub, sub, ES, elem_step=STEP,
                                    single_packet=False,
                                )
                            colofs += sub // 128
                            sofs += sub // 16
                            done += sub
    nc.compile()
    return nc


def build(lr, ref, index_map):
    """Host prep: returns (nc, in_maps, assemble) without running."""
    ref = np.ascontiguousarray(np.asarray(ref, dtype=np.float32))
    index_map = np.asarray(index_map)

    padded = np.pad(ref, ((0, 0), (0, 0), (1, 1), (1, 1)), mode="reflect")
    imgs = [
        np.ascontiguousarray(
            padded[b].transpose(1, 2, 0).reshape(NPIX, C))
        for b in range(B)
    ]

    caps, gidx_cores, sidx_cores = _build_streams(index_map)
    nc = _build_program(caps)

    in_maps = [
        {
            "img": imgs[k // 4],
            "gidx": gidx_cores[k],
            "sidx": sidx_cores[k],
        }
        for k in range(N_CORES)
    ]

    def assemble(results):
        out = np.empty((B, HO, WO, C), np.float32)
        for k in range(N_CORES):
            b, r0 = k // 4, (k % 4) * SLAB
            out[b, r0:r0 + SLAB] = results[k]["out"][:SLAB]
        return np.ascontiguousarray(out.transpose(0, 3, 1, 2))

    return nc, in_maps, assemble


def kernel(lr, ref, index_map):
    # Run in two 4-core batches: 8 concurrent SWDGE-heavy cores overload
    # the DMA rings (device becomes unrecoverable); 4 are stable.  The
    # program is SPMD with no collectives, so batch = just another launch.
    from concourse.bass_utils import run_bass_kernel_spmd

    nc, in_maps, assemble = build(lr, ref, index_map)
    results = []
    for lo in range(0, N_CORES, CORES_PER_LAUNCH):
        ncore = min(CORES_PER_LAUNCH, N_CORES - lo)
        res = run_bass_kernel_spmd(
            nc, in_maps[lo:lo + ncore], list(range(ncore)))
        results.extend(res.results)
    return assemble(results)



# revision 6
# speedup vs baseline: 2.9404x; 2.9404x over previous
"""CoarseWarp Trainium2 kernel (v2: gather + on-chip fold, no scatter).

Reference computation (shapes hardcoded):
  lr [2,64,64,64] (shape-only), ref [2,64,254,254], index_map [2,64516]
  padded = reflect-pad(ref, 1)                      # [2,64,256,256]
  (yp, xp) = divmod(index_map[b, y*254+x], 254)
  out[b,c,y+i,x+j] += padded[b,c,yp+i,xp+j]  (i,j in 0..2)
  out: [2,64,256,256] f32

Strategy (8 NeuronCores, pure data parallel):
  core k handles b = k//4, output rows [ (k%4)*64, +64 ).

  Host builds, per batch, a fp16 patch table anchored at even x':
    T[1 + r*127 + e] = padded[r:r+3, 2e:2e+4, :]    # [i(3), u(4), c(64)]
  (entry 0 = zeros, used to pad streams) so each output position l
  needs ONE 1536 B dma_gather element: idx = 1 + yp*127 + xp//2, and
  the intra-element x-offset is phi = xp%2, supplied as a dense fp16
  mask stream.  Tap j reads u = phi + j, realized on-chip as 3
  copy_predicated selects (u[v] := phi ? u[v+1] : u[v]).

  The fold is deterministic: gathered slot x lands at partition x%128,
  col x//128; contribution (i, v) adds G[x][i, v] into out[y+i, x+v].
  Per 16-row block, 9 partition-aligned DVE adds accumulate three
  per-v accumulators O_v; the x+v partition shift is applied by tiny
  SBUF->SBUF DMA copies, then the three planes merge into f32 and are
  written out sequentially (16 KB descriptors, X-major HBM layout that
  the host assembles back).
"""

import numpy as np

B, C = 2, 64
HR = WR = 254
HO = WO = 256
L = HR * WR            # 64516
NTAB = 1 + HR * 127    # 32259 table entries (entry 0 = zeros)
ES = 768               # fp16 elems per entry: 3*4*64 (1536 B)
N_CORES = 8
CORES_PER_LAUNCH = 4
SLAB = 64              # output rows per core
NBLK = 4
NY = SLAB // NBLK      # 16 output rows per block
NG = NY + 2            # 18 gathered l-rows per block
NCOL = NG * 2          # 36 G4 cols per block
NIDX_BLK = NG * 256    # 4608 gather indices per block
MAXI = 1280            # max indices per SWDGE instruction


def _wrap16(stream: np.ndarray) -> np.ndarray:
    """Index stream [N] (N%16==0) -> ucode layout [128, N//16] int16."""
    a = stream.reshape(-1, 16).T.astype(np.int16)
    return np.tile(a, (8, 1))


def _build_tables(ref: np.ndarray) -> list[np.ndarray]:
    """Per-batch fp16 patch tables [NTAB, ES]."""
    padded = np.pad(ref.astype(np.float32),
                    ((0, 0), (0, 0), (1, 1), (1, 1)), mode="reflect")
    tabs = []
    for b in range(B):
        p = np.ascontiguousarray(
            padded[b].transpose(1, 2, 0)).astype(np.float16)  # [256,256,64]
        T = np.zeros((NTAB, ES), np.float16)
        V = T[1:].reshape(HR, 127, 3, 4, C)
        for i in range(3):
            for u in range(4):
                V[:, :, i, u, :] = p[i:i + HR, u:u + 254:2, :][:, :127, :]
        tabs.append(T)
    return tabs


def _build_streams(index_map: np.ndarray):
    """Per-core gather idx [128, NBLK*NIDX_BLK//16] i16 and phi mask
    [128, NBLK*NCOL] f16 streams."""
    index_map = np.asarray(index_map).astype(np.int64)
    yp = index_map // WR
    xp = index_map % WR
    idx_full = (1 + yp * 127 + (xp >> 1)).reshape(B, HR, WR)
    phi_full = (xp & 1).reshape(B, HR, WR)

    gidx_cores, mask_cores = [], []
    for k in range(N_CORES):
        b, r0 = k // 4, (k % 4) * SLAB
        idx_blks = np.zeros((NBLK, NG, 256), np.int64)
        phi_blks = np.zeros((NBLK, NG, 256), np.uint8)
        for blk in range(NBLK):
            ys = r0 + blk * NY - 2 + np.arange(NG)
            valid = (ys >= 0) & (ys < HR)
            idx_blks[blk, valid, :WR] = idx_full[b, ys[valid], :]
            phi_blks[blk, valid, :WR] = phi_full[b, ys[valid], :]
        gidx_cores.append(_wrap16(idx_blks.reshape(-1)))
        # M[p, blk*NCOL + g*2 + xc] = phi[blk, g, xc*128+p]
        M = phi_blks.reshape(NBLK, NG, 2, 128).transpose(3, 0, 1, 2)
        mask_cores.append(np.ascontiguousarray(M.reshape(128, NBLK * NCOL)))
    return gidx_cores, mask_cores


def _build_program():
    """Build the SPMD Bacc program (shared by all 8 cores)."""
    import bass_rust
    import concourse.bacc as bacc
    import concourse.tile as tile
    from concourse import mybir
    from concourse.library_config import mlp

    GCOLS = NBLK * NIDX_BLK // 16   # 1152
    MCOLS = NBLK * NCOL             # 144

    nc = bacc.Bacc(
        "TRN2",
        target_bir_lowering=False,
        debug=False,
        enable_asserts=False,
        num_devices=N_CORES,
    )
    tab_t = nc.dram_tensor(
        "tab", [NTAB, ES], mybir.dt.float16, kind="ExternalInput")
    gidx_t = nc.dram_tensor(
        "gidx", [128, GCOLS], mybir.dt.int16, kind="ExternalInput")
    mask_t = nc.dram_tensor(
        "mask", [128, MCOLS], mybir.dt.uint8, kind="ExternalInput")
    out_t = nc.dram_tensor(
        "out", [2, 128, SLAB, C], mybir.dt.float32, kind="ExternalOutput")

    src_ap = tab_t[:, :].copy()
    src_ap.ap = bass_rust.VecI64Pair([[ES, NTAB], [1, ES]])

    with tile.TileContext(nc) as tc:
        with tc.tile_pool(name="idx", bufs=1) as idxpool, \
             tc.tile_pool(name="g4", bufs=2) as gpool, \
             tc.tile_pool(name="acc", bufs=2) as apool, \
             tc.tile_pool(name="mrg", bufs=2) as mpool:
            nc.gpsimd.load_library(mlp)

            GI = idxpool.tile([128, GCOLS], mybir.dt.int16)
            nc.sync.dma_start(out=GI[:], in_=gidx_t[:, :])
            M = idxpool.tile([128, MCOLS], mybir.dt.uint8)
            nc.sync.dma_start(out=M[:], in_=mask_t[:, :])

            for blk in range(NBLK):
                G4 = gpool.tile([128, NCOL, ES], mybir.dt.float16)
                done = 0
                while done < NIDX_BLK:
                    sub = min(MAXI, NIDX_BLK - done)
                    nc.gpsimd.dma_gather(
                        G4[:, done // 128:(done + sub) // 128, :],
                        src_ap,
                        GI[:, (blk * NIDX_BLK + done) // 16:
                           (blk * NIDX_BLK + done + sub) // 16],
                        sub, sub, ES, elem_step=ES,
                        single_packet=False,
                    )
                    done += sub

                # phi-compaction: u[v] := phi ? u[v+1] : u[v]  (v ascending)
                mask_ap = M[:, blk * NCOL:(blk + 1) * NCOL].copy()
                mask_ap.ap = bass_rust.VecI64Pair(
                    [[MCOLS, 128], [1, NCOL], [0, C]])
                for v in range(3):
                    for i in range(3):
                        out_ap = G4[:, :, :].copy()
                        out_ap.ap = bass_rust.VecI64Pair(
                            [[NCOL * ES, 128], [ES, NCOL], [1, C]])
                        out_ap.offset += i * 256 + v * C
                        dat_ap = G4[:, :, :].copy()
                        dat_ap.ap = bass_rust.VecI64Pair(
                            [[NCOL * ES, 128], [ES, NCOL], [1, C]])
                        dat_ap.offset += i * 256 + (v + 1) * C
                        nc.vector.copy_predicated(out_ap, mask_ap, dat_ap)

                # fold: O_v[:, xc, Yl, c] += G4[:, (Yl+2-i)*2+xc, i, v, c]
                Ovs = []
                for v in range(3):
                    Ov = apool.tile([128, 2, NY, C], mybir.dt.float16)
                    nc.vector.memset(Ov[:], 0.0)
                    Ovs.append(Ov)
                for v in range(3):
                    for i in range(3):
                        in_ap = G4[:, :, :].copy()
                        in_ap.ap = bass_rust.VecI64Pair(
                            [[NCOL * ES, 128], [2 * ES, NY], [ES, 2], [1, C]])
                        in_ap.offset += (2 - i) * 2 * ES + i * 256 + v * C
                        out_ap = Ovs[v][:, :, :, :].copy()
                        out_ap.ap = bass_rust.VecI64Pair(
                            [[2 * NY * C, 128], [C, NY], [NY * C, 2], [1, C]])
                        nc.vector.tensor_tensor(
                            out_ap, out_ap, in_ap, mybir.AluOpType.add)

                # x+v partition shift via SBUF->SBUF DMA, then f32 merge
                S1 = mpool.tile([128, 2, NY, C], mybir.dt.float16)
                S2 = mpool.tile([128, 2, NY, C], mybir.dt.float16)
                nc.vector.memset(S1[:], 0.0)
                nc.vector.memset(S2[:], 0.0)
                nc.sync.dma_start(
                    out=S1[1:128, :, :, :], in_=Ovs[1][0:127, :, :, :])
                nc.sync.dma_start(
                    out=S1[0:1, 1, :, :], in_=Ovs[1][127:128, 0, :, :])
                nc.sync.dma_start(
                    out=S2[2:128, :, :, :], in_=Ovs[2][0:126, :, :, :])
                nc.sync.dma_start(
                    out=S2[0:2, 1, :, :], in_=Ovs[2][126:128, 0, :, :])

                O32 = mpool.tile([128, 2, NY, C], mybir.dt.float32)
                nc.vector.tensor_tensor(
                    O32[:, :, :, :], Ovs[0][:, :, :, :], S1[:, :, :, :],
                    mybir.AluOpType.add)
                nc.vector.tensor_tensor(
                    O32[:, :, :, :], O32[:, :, :, :], S2[:, :, :, :],
                    mybir.AluOpType.add)

                # writeout: out_t[xc, p, blk*NY+Yl, c] = O32[p, xc, Yl, c]
                out_ap = out_t[:, :, :, :].copy()
                out_ap.ap = bass_rust.VecI64Pair(
                    [[SLAB * C, 128], [128 * SLAB * C, 2], [C, NY], [1, C]])
                out_ap.offset += blk * NY * C
                nc.sync.dma_start(out=out_ap, in_=O32[:, :, :, :])
    nc.compile()
    return nc


def build(lr, ref, index_map):
    """Host prep: returns (nc, in_maps, assemble) without running."""
    ref = np.ascontiguousarray(np.asarray(ref, dtype=np.float32))
    tabs = _build_tables(ref)
    gidx_cores, mask_cores = _build_streams(index_map)
    nc = _build_program()

    in_maps = [
        {
            "tab": tabs[k // 4],
            "gidx": gidx_cores[k],
            "mask": mask_cores[k],
        }
        for k in range(N_CORES)
    ]

    def assemble(results):
        out = np.empty((B, C, HO, WO), np.float32)
        for k in range(N_CORES):
            b, r0 = k // 4, (k % 4) * SLAB
            arr = results[k]["out"]        # [2, 128, SLAB, C]
            # out[b, c, r0+Yl, xc*128+p] = arr[xc, p, Yl, c]
            out[b, :, r0:r0 + SLAB, :] = (
                arr.transpose(3, 2, 0, 1).reshape(C, SLAB, HO))
        return out

    return nc, in_maps, assemble


def kernel(lr, ref, index_map):
    from concourse.bass_utils import run_bass_kernel_spmd

    nc, in_maps, assemble = build(lr, ref, index_map)
    results = []
    for lo in range(0, N_CORES, CORES_PER_LAUNCH):
        ncore = min(CORES_PER_LAUNCH, N_CORES - lo)
        res = run_bass_kernel_spmd(
            nc, in_maps[lo:lo + ncore], list(range(ncore)))
        results.extend(res.results)
    return assemble(results)


# revision 7
# speedup vs baseline: 5.7754x; 1.9641x over previous
"""CoarseWarp Trainium2 kernel (v2: gather + on-chip fold, no scatter).

Reference computation (shapes hardcoded):
  lr [2,64,64,64] (shape-only), ref [2,64,254,254], index_map [2,64516]
  padded = reflect-pad(ref, 1)                      # [2,64,256,256]
  (yp, xp) = divmod(index_map[b, y*254+x], 254)
  out[b,c,y+i,x+j] += padded[b,c,yp+i,xp+j]  (i,j in 0..2)
  out: [2,64,256,256] f32

Strategy (8 NeuronCores, pure data parallel):
  core k handles b = k//4, output rows [ (k%4)*64, +64 ).

  Host builds, per batch, a fp16 patch table anchored at even x':
    T[1 + r*127 + e] = padded[r:r+3, 2e:2e+4, :]    # [i(3), u(4), c(64)]
  (entry 0 = zeros, used to pad streams) so each output position l
  needs ONE 1536 B dma_gather element: idx = 1 + yp*127 + xp//2, and
  the intra-element x-offset is phi = xp%2, supplied as a dense fp16
  mask stream.  Tap j reads u = phi + j, realized on-chip as 3
  copy_predicated selects (u[v] := phi ? u[v+1] : u[v]).

  The fold is deterministic: gathered slot x lands at partition x%128,
  col x//128; contribution (i, v) adds G[x][i, v] into out[y+i, x+v].
  Per 16-row block, 9 partition-aligned DVE adds accumulate three
  per-v accumulators O_v; the x+v partition shift is applied by tiny
  SBUF->SBUF DMA copies, then the three planes merge into f32 and are
  written out sequentially (16 KB descriptors, X-major HBM layout that
  the host assembles back).
"""

import numpy as np

B, C = 2, 64
HR = WR = 254
HO = WO = 256
L = HR * WR            # 64516
NTAB = 1 + HR * 127    # 32259 table entries (entry 0 = zeros)
ES = 768               # fp16 elems per entry: 3*4*64 (1536 B)
N_CORES = 8
CORES_PER_LAUNCH = 8
SLAB = 64              # output rows per core
NBLK = 4
NY = SLAB // NBLK      # 16 output rows per block
NG = NY + 2            # 18 gathered l-rows per block
NCOL = NG * 2          # 36 G4 cols per block
NIDX_BLK = NG * 256    # 4608 gather indices per block
MAXI = 1280            # max indices per SWDGE instruction


def _wrap16(stream: np.ndarray) -> np.ndarray:
    """Index stream [N] (N%16==0) -> ucode layout [128, N//16] int16."""
    a = stream.reshape(-1, 16).T.astype(np.int16)
    return np.tile(a, (8, 1))


def _build_tables(ref: np.ndarray) -> list[np.ndarray]:
    """Per-batch fp16 patch tables [NTAB, ES]."""
    padded = np.pad(ref.astype(np.float32),
                    ((0, 0), (0, 0), (1, 1), (1, 1)), mode="reflect")
    tabs = []
    for b in range(B):
        p = np.ascontiguousarray(
            padded[b].transpose(1, 2, 0)).astype(np.float16)  # [256,256,64]
        T = np.zeros((NTAB, ES), np.float16)
        V = T[1:].reshape(HR, 127, 3, 4, C)
        for i in range(3):
            for u in range(4):
                V[:, :, i, u, :] = p[i:i + HR, u:u + 254:2, :][:, :127, :]
        tabs.append(T)
    return tabs


def _build_streams(index_map: np.ndarray):
    """Per-core gather idx [128, NBLK*NIDX_BLK//16] i16 and phi mask
    [128, NBLK*NCOL] f16 streams."""
    index_map = np.asarray(index_map).astype(np.int64)
    yp = index_map // WR
    xp = index_map % WR
    idx_full = (1 + yp * 127 + (xp >> 1)).reshape(B, HR, WR)
    phi_full = (xp & 1).reshape(B, HR, WR)

    gidx_cores, mask_cores = [], []
    for k in range(N_CORES):
        b, r0 = k // 4, (k % 4) * SLAB
        idx_blks = np.zeros((NBLK, NG, 256), np.int64)
        phi_blks = np.zeros((NBLK, NG, 256), np.uint8)
        for blk in range(NBLK):
            ys = r0 + blk * NY - 2 + np.arange(NG)
            valid = (ys >= 0) & (ys < HR)
            idx_blks[blk, valid, :WR] = idx_full[b, ys[valid], :]
            phi_blks[blk, valid, :WR] = phi_full[b, ys[valid], :]
        gidx_cores.append(_wrap16(idx_blks.reshape(-1)))
        # M[p, blk*NCOL + g*2 + xc] = phi[blk, g, xc*128+p]
        M = phi_blks.reshape(NBLK, NG, 2, 128).transpose(3, 0, 1, 2)
        mask_cores.append(np.ascontiguousarray(M.reshape(128, NBLK * NCOL)))
    return gidx_cores, mask_cores


def _build_program():
    """Build the SPMD Bacc program (shared by all 8 cores)."""
    import bass_rust
    import concourse.bacc as bacc
    import concourse.tile as tile
    from concourse import mybir
    from concourse.library_config import mlp

    GCOLS = NBLK * NIDX_BLK // 16   # 1152
    MCOLS = NBLK * NCOL             # 144

    nc = bacc.Bacc(
        "TRN2",
        target_bir_lowering=False,
        debug=False,
        enable_asserts=False,
        num_devices=N_CORES,
    )
    tab_t = nc.dram_tensor(
        "tab", [NTAB, ES], mybir.dt.float16, kind="ExternalInput")
    gidx_t = nc.dram_tensor(
        "gidx", [128, GCOLS], mybir.dt.int16, kind="ExternalInput")
    mask_t = nc.dram_tensor(
        "mask", [128, MCOLS], mybir.dt.uint8, kind="ExternalInput")
    out_t = nc.dram_tensor(
        "out", [2, 128, SLAB, C], mybir.dt.float32, kind="ExternalOutput")

    src_ap = tab_t[:, :].copy()
    src_ap.ap = bass_rust.VecI64Pair([[ES, NTAB], [1, ES]])

    with tile.TileContext(nc) as tc:
        with tc.tile_pool(name="idx", bufs=1) as idxpool, \
             tc.tile_pool(name="g4", bufs=2) as gpool, \
             tc.tile_pool(name="acc", bufs=2) as apool, \
             tc.tile_pool(name="mrg", bufs=2) as mpool:
            nc.gpsimd.load_library(mlp)

            GI = idxpool.tile([128, GCOLS], mybir.dt.int16)
            nc.sync.dma_start(out=GI[:], in_=gidx_t[:, :])
            M = idxpool.tile([128, MCOLS], mybir.dt.uint8)
            nc.sync.dma_start(out=M[:], in_=mask_t[:, :])

            for blk in range(NBLK):
                G4 = gpool.tile([128, NCOL, ES], mybir.dt.float16)
                done = 0
                while done < NIDX_BLK:
                    sub = min(MAXI, NIDX_BLK - done)
                    nc.gpsimd.dma_gather(
                        G4[:, done // 128:(done + sub) // 128, :],
                        src_ap,
                        GI[:, (blk * NIDX_BLK + done) // 16:
                           (blk * NIDX_BLK + done + sub) // 16],
                        sub, sub, ES, elem_step=ES,
                        single_packet=False,
                    )
                    done += sub

                # phi-compaction: u[v] := phi ? u[v+1] : u[v]  (v ascending)
                mask_ap = M[:, blk * NCOL:(blk + 1) * NCOL].copy()
                mask_ap.ap = bass_rust.VecI64Pair(
                    [[MCOLS, 128], [1, NCOL], [0, C]])
                for v in range(3):
                    for i in range(3):
                        out_ap = G4[:, :, :].copy()
                        out_ap.ap = bass_rust.VecI64Pair(
                            [[NCOL * ES, 128], [ES, NCOL], [1, C]])
                        out_ap.offset += i * 256 + v * C
                        dat_ap = G4[:, :, :].copy()
                        dat_ap.ap = bass_rust.VecI64Pair(
                            [[NCOL * ES, 128], [ES, NCOL], [1, C]])
                        dat_ap.offset += i * 256 + (v + 1) * C
                        nc.vector.copy_predicated(out_ap, mask_ap, dat_ap)

                # fold: O_v[:, xc, Yl, c] += G4[:, (Yl+2-i)*2+xc, i, v, c]
                Ovs = []
                for v in range(3):
                    Ov = apool.tile([128, 2, NY, C], mybir.dt.float16)
                    nc.vector.memset(Ov[:], 0.0)
                    Ovs.append(Ov)
                for v in range(3):
                    for i in range(3):
                        in_ap = G4[:, :, :].copy()
                        in_ap.ap = bass_rust.VecI64Pair(
                            [[NCOL * ES, 128], [2 * ES, NY], [ES, 2], [1, C]])
                        in_ap.offset += (2 - i) * 2 * ES + i * 256 + v * C
                        out_ap = Ovs[v][:, :, :, :].copy()
                        out_ap.ap = bass_rust.VecI64Pair(
                            [[2 * NY * C, 128], [C, NY], [NY * C, 2], [1, C]])
                        nc.vector.tensor_tensor(
                            out_ap, out_ap, in_ap, mybir.AluOpType.add)

                # x+v partition shift via SBUF->SBUF DMA, then f32 merge
                S1 = mpool.tile([128, 2, NY, C], mybir.dt.float16)
                S2 = mpool.tile([128, 2, NY, C], mybir.dt.float16)
                nc.vector.memset(S1[:], 0.0)
                nc.vector.memset(S2[:], 0.0)
                nc.sync.dma_start(
                    out=S1[1:128, :, :, :], in_=Ovs[1][0:127, :, :, :])
                nc.sync.dma_start(
                    out=S1[0:1, 1, :, :], in_=Ovs[1][127:128, 0, :, :])
                nc.sync.dma_start(
                    out=S2[2:128, :, :, :], in_=Ovs[2][0:126, :, :, :])
                nc.sync.dma_start(
                    out=S2[0:2, 1, :, :], in_=Ovs[2][126:128, 0, :, :])

                O32 = mpool.tile([128, 2, NY, C], mybir.dt.float32)
                nc.vector.tensor_tensor(
                    O32[:, :, :, :], Ovs[0][:, :, :, :], S1[:, :, :, :],
                    mybir.AluOpType.add)
                nc.vector.tensor_tensor(
                    O32[:, :, :, :], O32[:, :, :, :], S2[:, :, :, :],
                    mybir.AluOpType.add)

                # writeout: out_t[xc, p, blk*NY+Yl, c] = O32[p, xc, Yl, c]
                out_ap = out_t[:, :, :, :].copy()
                out_ap.ap = bass_rust.VecI64Pair(
                    [[SLAB * C, 128], [128 * SLAB * C, 2], [C, NY], [1, C]])
                out_ap.offset += blk * NY * C
                nc.sync.dma_start(out=out_ap, in_=O32[:, :, :, :])
    nc.compile()
    return nc


def build(lr, ref, index_map):
    """Host prep: returns (nc, in_maps, assemble) without running."""
    ref = np.ascontiguousarray(np.asarray(ref, dtype=np.float32))
    tabs = _build_tables(ref)
    gidx_cores, mask_cores = _build_streams(index_map)
    nc = _build_program()

    in_maps = [
        {
            "tab": tabs[k // 4],
            "gidx": gidx_cores[k],
            "mask": mask_cores[k],
        }
        for k in range(N_CORES)
    ]

    def assemble(results):
        out = np.empty((B, C, HO, WO), np.float32)
        for k in range(N_CORES):
            b, r0 = k // 4, (k % 4) * SLAB
            arr = results[k]["out"]        # [2, 128, SLAB, C]
            # out[b, c, r0+Yl, xc*128+p] = arr[xc, p, Yl, c]
            out[b, :, r0:r0 + SLAB, :] = (
                arr.transpose(3, 2, 0, 1).reshape(C, SLAB, HO))
        return out

    return nc, in_maps, assemble


def kernel(lr, ref, index_map):
    from concourse.bass_utils import run_bass_kernel_spmd

    nc, in_maps, assemble = build(lr, ref, index_map)
    results = []
    for lo in range(0, N_CORES, CORES_PER_LAUNCH):
        ncore = min(CORES_PER_LAUNCH, N_CORES - lo)
        res = run_bass_kernel_spmd(
            nc, in_maps[lo:lo + ncore], list(range(ncore)))
        results.extend(res.results)
    return assemble(results)


# revision 13
# speedup vs baseline: 9.8397x; 1.7037x over previous
"""CoarseWarp Trainium2 kernel (v2: gather + on-chip fold, no scatter).

Reference computation (shapes hardcoded):
  lr [2,64,64,64] (shape-only), ref [2,64,254,254], index_map [2,64516]
  padded = reflect-pad(ref, 1)                      # [2,64,256,256]
  (yp, xp) = divmod(index_map[b, y*254+x], 254)
  out[b,c,y+i,x+j] += padded[b,c,yp+i,xp+j]  (i,j in 0..2)
  out: [2,64,256,256] f32

Strategy (8 NeuronCores, pure data parallel):
  core k handles b = k//4, output rows [ (k%4)*64, +64 ).

  Host builds, per batch, a fp16 patch table anchored at even x':
    T[1 + r*127 + e] = padded[r:r+3, 2e:2e+4, :]    # [i(3), u(4), c(64)]
  (entry 0 = zeros, used to pad streams) so each output position l
  needs ONE 1536 B dma_gather element: idx = 1 + yp*127 + xp//2, and
  the intra-element x-offset is phi = xp%2, supplied as a dense fp16
  mask stream.  Tap j reads u = phi + j, realized on-chip as 3
  copy_predicated selects (u[v] := phi ? u[v+1] : u[v]).

  The fold is deterministic: gathered slot x lands at partition x%128,
  col x//128; contribution (i, v) adds G[x][i, v] into out[y+i, x+v].
  Per 16-row block the fold runs on the (otherwise idle) Tensor
  engine: the x+v partition shift is a banded stationary matrix
  (slices of one host-built [128,264] fp16 tile: shift-by-v plus a
  wrap matrix for the 128-boundary crossing), and the 9 (i,v) terms
  accumulate in PSUM (f32) per 512-element chunk.  Scalar copies
  PSUM->SBUF and the result is written out sequentially (16 KB
  descriptors, X-major HBM layout that the host assembles back).
"""

import numpy as np

B, C = 2, 64
HR = WR = 254
HO = WO = 256
L = HR * WR            # 64516
NTAB = 1 + HR * 127    # 32259 table entries (entry 0 = zeros)
ES = 768               # fp16 elems per entry: 3*4*64 (1536 B)
N_CORES = 8
CORES_PER_LAUNCH = 8
SLAB = 64              # output rows per core
NBLK = 4
NY = SLAB // NBLK      # 16 output rows per block
NG = NY + 2            # 18 gathered l-rows per block
NCOL = NG * 2          # 36 G4 cols per block
NIDX_BLK = NG * 256    # 4608 gather indices per block
MAXI = 1280            # max indices per SWDGE instruction


def _wrap16(stream: np.ndarray) -> np.ndarray:
    """Index stream [N] (N%16==0) -> ucode layout [128, N//16] int16."""
    a = stream.reshape(-1, 16).T.astype(np.int16)
    return np.tile(a, (8, 1))


def _build_tables(ref: np.ndarray) -> list[np.ndarray]:
    """Per-batch fp16 patch tables [NTAB, ES]."""
    padded = np.pad(ref.astype(np.float32),
                    ((0, 0), (0, 0), (1, 1), (1, 1)), mode="reflect")
    tabs = []
    for b in range(B):
        p = np.ascontiguousarray(
            padded[b].transpose(1, 2, 0)).astype(np.float16)  # [256,256,64]
        T = np.zeros((NTAB, ES), np.float16)
        V = T[1:].reshape(HR, 127, 3, 4, C)
        for i in range(3):
            for u in range(4):
                V[:, :, i, u, :] = p[i:i + HR, u:u + 254:2, :][:, :127, :]
        tabs.append(T)
    return tabs


def _build_streams(index_map: np.ndarray):
    """Per-core gather idx [128, NBLK*NIDX_BLK//16] i16 and phi mask
    [128, NBLK*NCOL] f16 streams."""
    index_map = np.asarray(index_map).astype(np.int64)
    yp = index_map // WR
    xp = index_map % WR
    idx_full = (1 + yp * 127 + (xp >> 1)).reshape(B, HR, WR)
    phi_full = (xp & 1).reshape(B, HR, WR)

    gidx_cores, mask_cores = [], []
    for k in range(N_CORES):
        b, r0 = k // 4, (k % 4) * SLAB
        idx_blks = np.zeros((NBLK, NG, 256), np.int64)
        phi_blks = np.zeros((NBLK, NG, 256), np.uint8)
        for blk in range(NBLK):
            ys = r0 + blk * NY - 2 + np.arange(NG)
            valid = (ys >= 0) & (ys < HR)
            idx_blks[blk, valid, :WR] = idx_full[b, ys[valid], :]
            phi_blks[blk, valid, :WR] = phi_full[b, ys[valid], :]
        gidx_cores.append(_wrap16(idx_blks.reshape(-1)))
        # M[p, blk*NCOL + g*2 + xc] = phi[blk, g, xc*128+p]
        M = phi_blks.reshape(NBLK, NG, 2, 128).transpose(3, 0, 1, 2)
        mask_cores.append(np.ascontiguousarray(M.reshape(128, NBLK * NCOL)))
    return gidx_cores, mask_cores


def _build_program():
    """Build the SPMD Bacc program (shared by all 8 cores)."""
    import bass_rust
    import concourse.bacc as bacc
    import concourse.tile as tile
    from concourse import mybir
    from concourse.library_config import mlp

    GCOLS = NBLK * NIDX_BLK // 16   # 1152
    MCOLS = NBLK * NCOL             # 144

    nc = bacc.Bacc(
        "TRN2",
        target_bir_lowering=False,
        debug=False,
        enable_asserts=False,
        num_devices=N_CORES,
    )
    tab_t = nc.dram_tensor(
        "tab", [NTAB, ES], mybir.dt.float16, kind="ExternalInput")
    gidx_t = nc.dram_tensor(
        "gidx", [128, GCOLS], mybir.dt.int16, kind="ExternalInput")
    mask_t = nc.dram_tensor(
        "mask", [128, MCOLS], mybir.dt.uint8, kind="ExternalInput")
    wts_t = nc.dram_tensor(
        "wts", [128, 264], mybir.dt.float16, kind="ExternalInput")
    out_t = nc.dram_tensor(
        "out", [2, 128, SLAB, C], mybir.dt.float32, kind="ExternalOutput")

    src_ap = tab_t[:, :].copy()
    src_ap.ap = bass_rust.VecI64Pair([[ES, NTAB], [1, ES]])

    with tile.TileContext(nc) as tc:
        with tc.tile_pool(name="idx", bufs=1) as idxpool, \
             tc.tile_pool(name="g4", bufs=2) as gpool, \
             tc.tile_pool(name="mrg", bufs=2) as mpool, \
             tc.psum_pool(name="ps", bufs=4) as ppool:
            nc.gpsimd.load_library(mlp)

            GI = idxpool.tile([128, GCOLS], mybir.dt.int16)
            nc.sync.dma_start(out=GI[:], in_=gidx_t[:, :])
            M = idxpool.tile([128, MCOLS], mybir.dt.uint8)
            nc.sync.dma_start(out=M[:], in_=mask_t[:, :])
            WT = idxpool.tile([128, 264], mybir.dt.float16)
            nc.sync.dma_start(out=WT[:], in_=wts_t[:, :])

            for blk in range(NBLK):
                G4 = gpool.tile([128, NCOL, ES], mybir.dt.float16)
                done = 0
                while done < NIDX_BLK:
                    sub = min(MAXI, NIDX_BLK - done)
                    nc.gpsimd.dma_gather(
                        G4[:, done // 128:(done + sub) // 128, :],
                        src_ap,
                        GI[:, (blk * NIDX_BLK + done) // 16:
                           (blk * NIDX_BLK + done + sub) // 16],
                        sub, sub, ES, elem_step=ES,
                        single_packet=False,
                    )
                    done += sub

                # phi-compaction: u[v] := phi ? u[v+1] : u[v]  (v ascending)
                mask_ap = M[:, blk * NCOL:(blk + 1) * NCOL].copy()
                mask_ap.ap = bass_rust.VecI64Pair(
                    [[MCOLS, 128], [1, NCOL], [0, C]])
                for v in range(3):
                    for i in range(3):
                        out_ap = G4[:, :, :].copy()
                        out_ap.ap = bass_rust.VecI64Pair(
                            [[NCOL * ES, 128], [ES, NCOL], [1, C]])
                        out_ap.offset += i * 256 + v * C
                        dat_ap = G4[:, :, :].copy()
                        dat_ap.ap = bass_rust.VecI64Pair(
                            [[NCOL * ES, 128], [ES, NCOL], [1, C]])
                        dat_ap.offset += i * 256 + (v + 1) * C
                        nc.vector.copy_predicated(out_ap, mask_ap, dat_ap)

                # fold on PE: out[q=x+v, Yl, c] = sum_iv SH_v @ G4[x, i, v]
                # accumulated in PSUM per 512-f32 chunk (xc, yh: 8 rows).
                O32 = mpool.tile([128, 2, NY, C], mybir.dt.float32)
                for xc in range(2):
                    for yh in range(2):
                        P = ppool.tile([128, 8 * C], mybir.dt.float32)
                        ops = []
                        for v in range(3):
                            ops += [(2 - v, 0, v, i) for i in range(3)]
                            if xc == 1 and v > 0:   # 128-boundary wrap
                                ops += [(132 + 4 - v, 1, v, i)
                                        for i in range(3)]
                        for n, (wofs, wrap, v, i) in enumerate(ops):
                            rhs = G4[:, :, :].copy()
                            rhs.ap = bass_rust.VecI64Pair(
                                [[NCOL * ES, 128], [2 * ES, 8], [1, C]])
                            rhs.offset += (
                                (2 - i + yh * 8) * 2 * ES
                                + (xc - wrap) * ES + i * 256 + v * C)
                            nc.tensor.matmul(
                                P[:, :], WT[:, wofs:wofs + 128], rhs,
                                start=(n == 0), stop=(n == len(ops) - 1))
                        nc.scalar.copy(
                            O32[:, xc, yh * 8:(yh + 1) * 8, :], P[:, :])

                # writeout: out_t[xc, p, blk*NY+Yl, c] = O32[p, xc, Yl, c]
                out_ap = out_t[:, :, :, :].copy()
                out_ap.ap = bass_rust.VecI64Pair(
                    [[SLAB * C, 128], [128 * SLAB * C, 2], [C, NY], [1, C]])
                out_ap.offset += blk * NY * C
                nc.sync.dma_start(out=out_ap, in_=O32[:, :, :, :])
    nc.compile()
    return nc


def _build_weights() -> np.ndarray:
    """[128, 264] f16: cols 0..131 banded shift (1 at j=p+2, so
    W_v = wts[:, 2-v:2-v+128] maps partition p -> p+v); cols 132..263
    wrap matrices (1 at j=132+p-124 for p>=124, so Wr_v =
    wts[:, 132+4-v:...] maps p -> p+v-128)."""
    W = np.zeros((128, 264), np.float16)
    p = np.arange(128)
    W[p, p + 2] = 1.0
    pw = np.arange(124, 128)
    W[pw, 132 + pw - 124] = 1.0
    return W


def build(lr, ref, index_map):
    """Host prep: returns (nc, in_maps, assemble) without running."""
    ref = np.ascontiguousarray(np.asarray(ref, dtype=np.float32))
    tabs = _build_tables(ref)
    gidx_cores, mask_cores = _build_streams(index_map)
    wts = _build_weights()
    nc = _build_program()

    in_maps = [
        {
            "tab": tabs[k // 4],
            "gidx": gidx_cores[k],
            "mask": mask_cores[k],
            "wts": wts,
        }
        for k in range(N_CORES)
    ]

    def assemble(results):
        out = np.empty((B, C, HO, WO), np.float32)
        for k in range(N_CORES):
            b, r0 = k // 4, (k % 4) * SLAB
            arr = results[k]["out"]        # [2, 128, SLAB, C]
            # out[b, c, r0+Yl, xc*128+p] = arr[xc, p, Yl, c]
            out[b, :, r0:r0 + SLAB, :] = (
                arr.transpose(3, 2, 0, 1).reshape(C, SLAB, HO))
        return out

    return nc, in_maps, assemble


def kernel(lr, ref, index_map):
    from concourse.bass_utils import run_bass_kernel_spmd

    nc, in_maps, assemble = build(lr, ref, index_map)
    results = []
    for lo in range(0, N_CORES, CORES_PER_LAUNCH):
        ncore = min(CORES_PER_LAUNCH, N_CORES - lo)
        res = run_bass_kernel_spmd(
            nc, in_maps[lo:lo + ncore], list(range(ncore)))
        results.extend(res.results)
    return assemble(results)
